# revision 53
# baseline (speedup 1.0000x reference)
"""Trainium2 Bass kernel for nn_Blur1: 3x3 cross blur + LIF neuron scan.

Reference semantics (per timestep t, state v/i per pixel):
    c    = conv2d_same(x[t], K)        # K = cross kernel (0.15 sides, 0.4 ctr)
    v_d  = 0.8*v + 0.2*i
    z[t] = (v_d - 1) > 0
    v    = (1-z)*v_d
    i    = 0.8*i + c

Strategy (8 NeuronCores = 4 H-shards x 2 W-shards, no collectives):
  * Scaled variables with s = k_ctr (0.4): c'' = c/s = 0.375*(u+d+l+r) + x_c,
    I'' = i/s, V'' = v/(0.2*s), TH = 12.5.  All conv coefficients (0.375, 1)
    are exact in fp16.
  * Conv entirely on the PE: per t-chunk of 8, 6 fp16 matmuls accumulate into
    PSUM: a tridiagonal vertical matrix and a 0.375-scaled identity applied
    to w-1 / w+1 shifted views, each against an exact hi/lo fp16 split of x
    (x = x_hi + x_lo, residual ~2^-22 -> 0 spike flips in numpy validation).
    fp16 matmuls run 4x the fp32 rate, so all five taps on PE beat a
    DVE/GPSIMD horizontal pass + merge (GPSIMD cannot access PSUM on TRN2).
  * Synaptic current I'': one DVE tensor_tensor_scan per t-chunk, reading
    PSUM directly, over a (w, t_local) slab; multiplier tile 0.8 zeroed at
    chunk starts.  The cross-chunk carry is injected by a small STT into the
    chunk's first t-column (reading the slab BEFORE halo accumulation).
    GPSIMD cannot run TensorScalarPtr ops or touch PSUM on TRN2, so the
    scan/fixup/V chain is DVE-only; chunk sizes [4,4]+[8]*14+[4,4] prime the
    pipeline, and a few high-priority warmup matmuls ramp the PE pstate.
  * Membrane V'': ONE custom DVE instruction per timestep (fused
    out = select(0.8*V + I < TH, 0.8*V + I, 0)), overwriting the consumed
    I slab slot in place, so slot t-1 ends holding V[t].
  * Spikes: z[t] <=> V[t] == 0 (v_dec == 0 exactly has measure ~0).  ACT
    Sign -> fp8 DMA out; host maps sign==0 to spike; z[0]=0 on host.
  * H-halo rows (partitions 0/127): host precomputes the FULL decayed scan
    of the neighbour-row contribution and SWDGE-accumulates each chunk's
    slice into the I slab after that chunk's scan.  The carry fixup is
    emitted before the previous chunk's halo accum, so the device carry
    chain never includes halo terms (host scan supplies them all) and the
    serial fixup->scan chain never waits on the DMA.
"""
import sys

for _p in ("/opt/trn_rl_repo",):
    if _p not in sys.path:
        sys.path.insert(0, _p)

import numpy as np
from concourse import bacc, mybir
import concourse.tile as tile
from concourse.bass_utils import run_bass_kernel_spmd

f32 = mybir.dt.float32
f16 = mybir.dt.float16
fp8 = mybir.dt.float8e4

T = 128          # timesteps
RPC = 128        # rows per core (H=512 / 4)
WPC = 256        # cols per core (W=512 / 2)
# chunk sizes: small first chunks prime the pipeline (scan/V start before the
# PE has ramped), small last chunks shrink the sign/DMA-out tail.
TCS = [4, 4] + [8] * 14 + [4, 4]
assert sum(TCS) == T
OFFS = [sum(TCS[:i]) for i in range(len(TCS))]
NCH = len(TCS)
DEC = 0.8
TH = 1.0 / (0.2 * 0.4)   # threshold in k_ctr-scaled units
KS = 0.375               # side tap / center tap

_CACHE = {}
TUNE = {"x_bufs": 3, "ps_bufs": 2, "zs_bufs": 4}
NPH = 0          # chunks whose halo rides the PE (rest: SWDGE accum)


def _register_lif_step():
    """LIF_STEP custom DVE op: out = select(y < C0, y, 0), y = Src0*C1 + Src1.
    One DVE instruction per membrane timestep instead of two STT passes."""
    import concourse.dve_ops as dve_ops
    from concourse.dve_spec import (C0, C1, Spec, Src0, Src1, Zero, select,
                                    lower, _has_src1)
    from concourse.dve_uop import DveOpSpec
    from concourse.dve_table_gen import dve_ver_for

    for op in dve_ops.OPS:
        if op.name == "LIF_STEP":
            return op

    y = Src0 * C1 + Src1

    def ref(in0, in1, c0, c1, c2):
        yv = (np.asarray(in0, np.float32) * c1
              + np.asarray(in1, np.float32)).astype(np.float32)
        return np.where(yv < c0, yv, np.float32(0.0)).astype(np.float32)

    spec = Spec(body=select(y < C0, y, Zero), reference=ref)
    name = "LIF_STEP"
    row = max(dve_ops._SUB_OPCODE_FOR_NAME.values()) + 1
    assert row < 0x20
    dve_ops._SUB_OPCODE_FOR_NAME[name] = row
    ver = dve_ver_for("TRN2")
    uops = lower(spec, ver=ver)
    probe = DveOpSpec(name=name, opcode=row, uops=uops, rd1_en=_has_src1(spec))
    op = dve_ops.DveOp(name, spec, subdim=False,
                       uops_sha={ver: probe.sha(ver)})
    dve_ops.OPS.append(op)
    dve_ops.CUSTOM_DVE_SPECS[name] = spec
    return op


def _build_cached():
    if "nc" not in _CACHE:
        _CACHE["nc"] = _build()
    return _CACHE["nc"]


def _build():
    LIF = _register_lif_step()
    nc = bacc.Bacc("TRN2", target_bir_lowering=False, debug=False,
                   num_devices=8)

    xhi = nc.declare_dram_parameter("xhi", [RPC, T, WPC + 2], f16, isOutput=False)
    xlo = nc.declare_dram_parameter("xlo", [RPC, T, WPC + 2], f16, isOutput=False)
    mv = nc.declare_dram_parameter("mv", [RPC, RPC], f16, isOutput=False)
    mh = nc.declare_dram_parameter("mh", [RPC, RPC], f16, isOutput=False)
    xh = nc.declare_dram_parameter("xh", [2, T * WPC], f32, isOutput=False)
    # raw neighbour rows for the first NPH chunks (halo via PE matmul)
    xph = nc.declare_dram_parameter("xph", [2, 2, max(OFFS[NPH], 1) * WPC], f16,
                                    isOutput=False)
    mph = nc.declare_dram_parameter("mph", [2, RPC], f16, isOutput=False)
    zo = nc.declare_dram_parameter("zo", [RPC, T, WPC], fp8, isOutput=True)
    d0d = {tc_: nc.declare_dram_parameter(f"d0_{tc_}", [128, WPC * tc_], f32,
                                          isOutput=False)
           for tc_ in sorted(set(TCS))}
    ztd = nc.declare_dram_parameter("ztd", [128, WPC], f32, isOutput=False)

    with tile.TileContext(nc) as tc:
        with tc.tile_pool(name="keep", bufs=1) as keep:
            mvt = keep.tile([RPC, RPC], f16)
            mht = keep.tile([RPC, RPC], f16)

            slab = keep.tile([128, WPC * T], f32)

            def cview(c):
                """[p, w, t_local] view of chunk c of the slab."""
                tc_, off = TCS[c], OFFS[c]
                return slab[:, off * WPC:(off + tc_) * WPC].rearrange(
                    "p (w t) -> p w t", t=tc_)

            d0s = {tc_: keep.tile([128, WPC * tc_], f32, name=f"d0t{tc_}")
                   for tc_ in sorted(set(TCS))}
            zt = keep.tile([128, WPC], f32)
            mpht = keep.tile([2, RPC], f16)
            xpht = keep.tile([2, 2 * max(OFFS[NPH], 1) * WPC], f16)

            with tc.high_priority():
                nc.sync.dma_start(mvt[:], mv[:])
                nc.sync.dma_start(mht[:], mh[:])
                if NPH:
                    nc.sync.dma_start(mpht[:], mph[:])
                    nc.sync.dma_start(xpht[:],
                                      xph[:].rearrange("h v f -> h (v f)"))
                for tc_, d in d0s.items():
                    nc.gpsimd.memset(d[:], DEC)
                    dv = d[:].rearrange("p (w t) -> p w t", t=tc_)
                    nc.gpsimd.memset(dv[:, :, 0:1], 0.0)
                nc.gpsimd.memset(zt[:], 0.0)

            # PE warmup: keep the tensor engine busy from t~0 so the first
            # real conv matmuls run at full pstate (ramp needs ~3us busy).
            # Feeds on the memset zt tile so it needs no DMA to start.
            with tc.tile_pool(name="wu", bufs=1, space="PSUM") as wup:
                wut = wup.tile([128, 128], f32)
                with tc.high_priority():
                    for _ in range(6):
                        nc.tensor.matmul(wut[:], mvt[:], mht[:],
                                         start=True, stop=True)

            def halo(c):
                """accumulate host-prescanned halo rows into slab chunk c.
                Chunks < NPH get their halo through the PE instead."""
                if c < NPH:
                    return
                tc_, off = TCS[c], OFFS[c]
                nc.gpsimd.dma_start(
                    cview(c)[0:128:127, :, :].rearrange("p w t -> p (w t)"),
                    xh[:, off * WPC:(off + tc_) * WPC],
                    accum_op=mybir.AluOpType.add)

            def tail(c, zsp):
                """V steps + sign + DMA out for chunk c."""
                tc_, off = TCS[c], OFFS[c]
                cv = cview(c)
                for t in range(off + 1, off + tc_ + 1):
                    if t > T - 1:
                        break
                    j = t - 1          # slab slot consumed & overwritten
                    cc = c if j >= off else c - 1
                    slot = cview(cc)[:, :, j - OFFS[cc]:j - OFFS[cc] + 1]
                    if t == 1:
                        in0 = zt[:]
                    else:
                        cp = cc if j - 1 >= OFFS[cc] else cc - 1
                        in0 = cview(cp)[:, :, j - 1 - OFFS[cp]:j - OFFS[cp]]
                    nc.vector._custom_dve(LIF, out=slot, in0=in0, in1=slot,
                                          s0=TH, s1=DEC)
                zst = zsp.tile([128, WPC * max(TCS)], fp8, tag="zst",
                               name="zst")[:, :WPC * tc_]
                zsv = zst.rearrange("p (t w) -> p w t", w=WPC)
                nc.scalar.activation(zsv, cv[:, :, :],
                                     mybir.ActivationFunctionType.Sign)
                nc.sync.dma_start(
                    zo[:, off:off + tc_, :].rearrange("p t w -> p (t w)"),
                    zst)

            with tc.tile_pool(name="xb", bufs=TUNE["x_bufs"]) as xbp, \
                 tc.tile_pool(name="ps", bufs=TUNE["ps_bufs"], space="PSUM") as psp, \
                 tc.tile_pool(name="zs", bufs=TUNE["zs_bufs"]) as zsp:
                for c in range(NCH):
                    tc_, t0 = TCS[c], OFFS[c]
                    # ---- front end: load + conv into psum ----
                    xht = xbp.tile([128, max(TCS) * (WPC + 2)], f16,
                                   tag="xh", name="xht")[:, :tc_ * (WPC + 2)]
                    xlt = xbp.tile([128, max(TCS) * (WPC + 2)], f16,
                                   tag="xl", name="xlt")[:, :tc_ * (WPC + 2)]
                    nc.sync.dma_start(
                        xht, xhi[:, t0:t0 + tc_, :].rearrange("p t w -> p (t w)"))
                    nc.sync.dma_start(
                        xlt, xlo[:, t0:t0 + tc_, :].rearrange("p t w -> p (t w)"))
                    xhv = xht.rearrange("p (t w) -> p t w", w=WPC + 2)
                    xlv = xlt.rearrange("p (t w) -> p t w", w=WPC + 2)

                    pst_t = psp.tile([128, WPC * max(TCS)], f32,
                                     tag="pst", name="pst")
                    pst = pst_t[:, :WPC * tc_]
                    pstv = pst.rearrange("p (w t) -> p w t", t=tc_)
                    plan = [(mvt, xhv, 1), (mvt, xlv, 1),
                            (mht, xhv, 0), (mht, xlv, 0),
                            (mht, xhv, 2), (mht, xlv, 2)]
                    xphv = xpht[:].rearrange("h (v f) -> h v f", v=2)
                    for m in range(4):
                        wg = m * 64
                        out = pst[:, wg * tc_:(wg + 64) * tc_]
                        mms = [(mat[:], xv[:, :, off_ + wg:off_ + wg + 64]
                                          .rearrange("p t w -> p w t"))
                               for mat, xv, off_ in plan]
                        if c < NPH:
                            base = OFFS[c] * WPC
                            sl = slice(base + wg * tc_, base + (wg + 64) * tc_)
                            mms += [(mpht[:], xphv[:, v, sl]) for v in (0, 1)]
                        for k, (lh, rh) in enumerate(mms):
                            nc.tensor.matmul(out, lh, rh, start=(k == 0),
                                             stop=(k == len(mms) - 1))

                    # V work for chunk c-2 goes ahead of this chunk's
                    # fixup+scan in the DVE queue: it fills the gap while
                    # the PE finishes this chunk's psum.
                    if c > 1:
                        tail(c - 2, zsp)

                    # ---- carry fixup + scan: c'' -> I slab (w-major) ----
                    sout = slab[:, t0 * WPC:(t0 + tc_) * WPC]
                    if c > 0:
                        pt = TCS[c - 1]
                        nc.vector.scalar_tensor_tensor(
                            pstv[:, :, 0:1], cview(c - 1)[:, :, pt - 1:pt],
                            DEC, pstv[:, :, 0:1],
                            mybir.AluOpType.mult, mybir.AluOpType.add)
                    # halo for the previous chunk: after the fixup's pre-halo
                    # read of the slab, with ~2 chunk-periods before tail(c-1)
                    # consumes it, so the SWDGE latency stays off the V chain.
                    if c > 0:
                        halo(c - 1)
                    nc.vector.tensor_tensor_scan(
                        sout, d0s[tc_][:], pst, 0.0,
                        mybir.AluOpType.mult, mybir.AluOpType.add)

                halo(NCH - 1)
                tail(NCH - 2, zsp)
                tail(NCH - 1, zsp)

    if not nc.is_finalized():
        nc.finalize()
    return nc


def kernel(x, kernel):
    x = np.asarray(x, dtype=np.float32)
    k = np.asarray(kernel, dtype=np.float32)[0, 0]   # [3,3]
    Tn, _, H, W = x.shape
    assert (Tn, H, W) == (T, 512, 512)
    s = float(k[1, 1])                    # center tap = 0.4
    assert abs(float(k[1, 0]) / s - KS) < 1e-6

    nc = _build_cached()

    # stationary matrices (fp16-exact entries)
    mvm = np.zeros((RPC, RPC), np.float16)
    for i in range(RPC):
        mvm[i, i] = 1.0
        if i + 1 < RPC:
            mvm[i, i + 1] = KS     # input row i feeds output row i+1's up-tap
            mvm[i + 1, i] = KS     # input row i+1 feeds output row i's down-tap
    mhm = (np.eye(RPC) * KS).astype(np.float16)

    xp = np.pad(x[:, 0], ((0, 0), (1, 1), (1, 1)))   # [T, 514, 514]

    in_maps = []
    for c8 in range(8):
        a, b = divmod(c8, 2)
        r0, w0 = RPC * a, WPC * b
        xs = np.ascontiguousarray(
            xp[:, 1 + r0:1 + r0 + RPC, w0:w0 + WPC + 2].transpose(1, 0, 2))
        xhi_ = xs.astype(np.float16)
        xlo_ = (xs - xhi_.astype(np.float32)).astype(np.float16)

        # halo neighbour rows.  Chunks < NPH: raw rows go through the PE
        # (fp16 hi/lo, same arithmetic as the main taps) and the device scan
        # propagates them, including across chunk carries.  Chunks >= NPH:
        # host prescan of contributions from t >= OFFS[NPH] only (the earlier
        # part is already carried on-device), SWDGE-accumulated post-scan.
        top = xp[:, r0, 1 + w0:1 + w0 + WPC].astype(np.float64)        # [T, W]
        bot = xp[:, 1 + r0 + RPC, 1 + w0:1 + w0 + WPC].astype(np.float64)
        t_ph = OFFS[NPH]
        xph_ = np.zeros((2, 2, max(t_ph, 1) * WPC), np.float16)
        xh_ = np.zeros((2, T, WPC), np.float32)
        for hb, row in ((0, top), (1, bot)):
            raw = row[:t_ph].astype(np.float32)                 # [t_ph, W]
            rhi = raw.astype(np.float16)
            rlo = (raw - rhi.astype(np.float32)).astype(np.float16)
            for v, rv in ((0, rhi), (1, rlo)):
                for c in range(NPH):
                    tc_, off = TCS[c], OFFS[c]
                    blk = rv[off:off + tc_, :].T               # [w, tl]
                    xph_[hb, v, off * WPC:(off + tc_) * WPC] = blk.reshape(-1)
            acc = np.zeros(WPC, np.float64)
            for t in range(t_ph, T):
                acc = DEC * acc + KS * row[t]
                xh_[hb, t] = acc.astype(np.float32)
        # pack [2, T, W] into per-chunk (w, t_local) blocks
        xhp = np.zeros((2, T * WPC), np.float32)
        for c, (tc_, off) in enumerate(zip(TCS, OFFS)):
            blk = xh_[:, off:off + tc_, :].transpose(0, 2, 1)   # [2, w, tl]
            xhp[:, off * WPC:(off + tc_) * WPC] = blk.reshape(2, -1)

        mphm = np.zeros((2, RPC), np.float16)
        mphm[0, 0] = KS
        mphm[1, RPC - 1] = KS

        im = {"xhi": xhi_, "xlo": xlo_, "mv": mvm, "mh": mhm, "xh": xhp,
              "xph": xph_, "mph": mphm,
              "ztd": np.zeros((128, WPC), np.float32)}
        for tc_ in sorted(set(TCS)):
            d = np.full((128, WPC, tc_), DEC, np.float32)
            d[:, :, 0] = 0.0
            im[f"d0_{tc_}"] = d.reshape(128, WPC * tc_)
        in_maps.append(im)

    res = run_bass_kernel_spmd(nc, in_maps, core_ids=list(range(8)))

    out = np.zeros((T, 1, H, W), np.float32)
    for c8 in range(8):
        a, b = divmod(c8, 2)
        s8 = np.asarray(res.results[c8]["zo"]).astype(np.float32)  # [p, t, w]
        zc = np.zeros((T, RPC, WPC), np.float32)
        zc[1:] = (s8[:, 0:T - 1, :] == 0.0).astype(np.float32).transpose(1, 0, 2)
        out[:, 0, RPC * a:RPC * (a + 1), WPC * b:WPC * (b + 1)] = zc
    return out
